# revision 1
# baseline (speedup 1.0000x reference)
"""Multi-head attention + output Linear on 8 Trainium2 NeuronCores.

Problem: bs=2, seq=2048, embed=1024, heads=16, head_dim=64.
  out = Linear(softmax(mask(Q K^T / 8)) V)        (eval-mode dropout)

Sharding: core c in 0..7 handles batch b = c//4 and query block qb = c%4
(512 query rows), computing its exact [512, 1024] output slice - heads stay
together per core so the output Linear needs no cross-core reduction.

Per-core kernel (Tile framework), all matmuls in float32r (1 cyc/row):
  scoresT[k, q] = K_h Q_h^T   (contraction over d=64, partition dim)
  probsT = exp(scoresT / 8) * maskT        (ACT exp fused scale; DVE/GPSIMD mul)
  outT[65, q]  = [V_h | 1]^T probsT        (ones column yields softmax denom)
  attnT = outT[0:64] * (1 / outT[64])      (denom broadcast via PE outer product)
  y = attnT^T W^T + bias                   (accumulate all 16 heads)

Heads are processed in pairs so K / V DMAs move 512B+ descriptors; the
[V | 1] lhsT tiles are assembled on-chip by DVE copies (tiny strided DMAs
were the dominant modeled cost).
"""

import sys
import numpy as np

sys.path.insert(0, "/opt/trn_rl_repo")

import concourse.bass as bass
import concourse.tile as tile
from concourse import bacc, mybir
from concourse.bass_utils import run_bass_kernel_spmd

BS, SEQ, EMBED, HEADS = 2, 2048, 1024, 16
D = EMBED // HEADS            # 64
QB = SEQ // 4                 # 512 query rows per core
NC_COUNT = 8
KC = SEQ // 128               # 16 k chunks
F32 = mybir.dt.float32
F32R = mybir.dt.float32r

_CACHE = {}


def _build_nc(scps_bufs=3, accps_bufs=2, probs_bufs=8, gps_mod=3, kpool_bufs=2,
              vpool_bufs=2, vapool_bufs=2, small_bufs=4, ypool_bufs=2,
              exp_group=2, interleave=False, il_probs_bufs=3):
    nc = bacc.Bacc("TRN2", target_bir_lowering=False, debug=False)

    qT = nc.dram_tensor("qT", [HEADS, D, QB], F32R, kind="ExternalInput")
    kT = nc.dram_tensor("kT", [HEADS, D, SEQ], F32R, kind="ExternalInput")
    vb = nc.dram_tensor("vb", [SEQ, EMBED], F32R, kind="ExternalInput")
    mT = nc.dram_tensor("mT", [SEQ, QB], mybir.dt.bfloat16, kind="ExternalInput")
    WT = nc.dram_tensor("WT", [EMBED, EMBED], F32R, kind="ExternalInput")
    bias = nc.dram_tensor("bias", [EMBED], F32, kind="ExternalInput")
    ones = nc.dram_tensor("ones", [128, D], F32R, kind="ExternalInput")
    y = nc.dram_tensor("y", [QB, EMBED], F32, kind="ExternalOutput")

    ngrp = KC // exp_group    # exp groups per head

    with tile.TileContext(nc) as tc, \
         nc.allow_low_precision(reason="float32r matmul inputs; fp32 accumulate in PSUM"):
        with tc.tile_pool(name="const", bufs=1) as const, \
             tc.tile_pool(name="kpool", bufs=kpool_bufs) as kpool, \
             tc.tile_pool(name="vpool", bufs=vpool_bufs) as vpool, \
             tc.tile_pool(name="vapool", bufs=vapool_bufs) as vapool, \
             tc.tile_pool(name="probs", bufs=probs_bufs) as probs, \
             tc.tile_pool(name="small", bufs=small_bufs) as small, \
             tc.tile_pool(name="ypool", bufs=ypool_bufs) as ypool, \
             tc.tile_pool(name="scps", bufs=scps_bufs, space="PSUM") as scps, \
             tc.tile_pool(name="accps", bufs=accps_bufs, space="PSUM") as accps:

            # ---- constants ----
            WT_sb = const.tile([128, 8, EMBED], F32R)
            nc.sync.dma_start(out=WT_sb, in_=WT.rearrange("(c p) e -> p c e", p=128))
            mT_sb = const.tile([128, KC, QB], mybir.dt.bfloat16)
            nc.sync.dma_start(out=mT_sb, in_=mT.rearrange("(c p) q -> p c q", p=128))
            qT_sb = const.tile([128, 8, QB], F32R)
            nc.sync.dma_start(
                out=qT_sb,
                in_=qT.rearrange("(hp two) d q -> (two d) hp q", two=2))
            bias_ap = bias[:]
            bias_bc = const.tile([128, EMBED], F32)
            nc.sync.dma_start(
                out=bias_bc,
                in_=bass.AP(tensor=bias_ap.tensor, offset=bias_ap.offset,
                            ap=[[0, 128]] + list(bias_ap.ap)),
            )
            ones_sb = const.tile([128, D], F32R)
            nc.sync.dma_start(out=ones_sb, in_=ones[:, :])
            attnT = const.tile([128, 8, QB], F32R)

            v_re = vb.rearrange("(c p) e -> p c e", p=128)

            for hp in range(8):            # head pairs
                kTp = kpool.tile([128, SEQ], F32R)
                nc.sync.dma_start(
                    out=kTp,
                    in_=kT[2 * hp:2 * hp + 2].rearrange("h d s -> (h d) s"))
                vp = vpool.tile([128, KC, 128], F32R)
                nc.sync.dma_start(out=vp,
                                  in_=v_re[:, :, hp * 128:(hp + 1) * 128])
                # assemble [V_h | 1] lhsT tiles on-chip (cheap DVE copies)
                va = vapool.tile([128, KC, 2, D + 1], F32R)
                nc.vector.tensor_copy(va[:, :, 0, 0:D], vp[:, :, 0:D])
                nc.vector.tensor_copy(va[:, :, 1, 0:D], vp[:, :, D:2 * D])
                nc.vector.tensor_copy(va[:, :, :, D], ones_sb[:, 0:KC * 2])

                if interleave:
                    outTs = [accps.tile([D + 1, QB], F32, tag="acc",
                                        name=f"outT{hp}_{hh}")
                             for hh in range(2)]
                    for g in range(8):
                        sc4 = scps.tile([128, 4, QB], F32, tag="sc")
                        for hh in range(2):
                            for j in range(2):
                                c = 2 * g + j
                                nc.tensor.matmul(
                                    sc4[:, 2 * hh + j, :],
                                    kTp[hh * D:hh * D + D, c * 128:(c + 1) * 128],
                                    qT_sb[hh * D:hh * D + D, hp, :],
                                    start=True, stop=True)
                        pe4 = probs.tile([128, 4, QB], F32R, tag="pe")
                        nc.scalar.activation(out=pe4, in_=sc4,
                                             func=mybir.ActivationFunctionType.Exp,
                                             scale=float(1.0 / np.sqrt(D)))
                        for hh in range(2):
                            eng = nc.gpsimd if (gps_mod and (2 * g + hh) % gps_mod == 0) else nc.vector
                            eng.tensor_mul(
                                pe4[:, 2 * hh:2 * hh + 2, :],
                                pe4[:, 2 * hh:2 * hh + 2, :],
                                mT_sb[:, 2 * g:2 * g + 2, :])
                        for hh in range(2):
                            for j in range(2):
                                c = 2 * g + j
                                nc.tensor.matmul(outTs[hh], va[:, c, hh, :],
                                                 pe4[:, 2 * hh + j, :],
                                                 start=(c == 0), stop=(c == KC - 1))
                    for hh in range(2):
                        outT = outTs[hh]
                        rc = small.tile([1, QB], F32R, tag="rc")
                        nc.vector.reciprocal(rc, outT[D:D + 1, :])
                        rb_ps = accps.tile([D, QB], F32, tag="acc")
                        nc.tensor.matmul(rb_ps, ones_sb[0:1, 0:D], rc[0:1, :],
                                         start=True, stop=True)
                        rb_sb = small.tile([D, QB], F32, tag="rb")
                        nc.vector.tensor_copy(rb_sb, rb_ps)
                        nc.vector.tensor_mul(
                            attnT[hh * D:hh * D + D, hp, :],
                            outT[0:D, :], rb_sb)
                    continue
                for hh in range(2):
                    h = 2 * hp + hh
                    outT = accps.tile([D + 1, QB], F32, tag="acc")
                    if exp_group == 3:
                        groups = [3, 3, 3, 3, 2, 2]
                    else:
                        groups = [exp_group] * (KC // exp_group)
                    c0 = 0
                    for g, gsz in enumerate(groups):
                        sc = scps.tile([128, gsz, QB], F32, tag="sc")
                        for j in range(gsz):
                            c = c0 + j
                            nc.tensor.matmul(
                                sc[:, j, :],
                                kTp[hh * D:hh * D + D, c * 128:(c + 1) * 128],
                                qT_sb[hh * D:hh * D + D, hp, :],
                                start=True, stop=True)
                        pe_t = probs.tile([128, gsz, QB], F32R, tag="pe")
                        nc.scalar.activation(out=pe_t, in_=sc,
                                             func=mybir.ActivationFunctionType.Exp,
                                             scale=float(1.0 / np.sqrt(D)))
                        eng = nc.gpsimd if (gps_mod and g % gps_mod == 0) else nc.vector
                        eng.tensor_mul(
                            pe_t, pe_t,
                            mT_sb[:, c0:c0 + gsz, :])
                        for j in range(gsz):
                            c = c0 + j
                            nc.tensor.matmul(outT, va[:, c, hh, :], pe_t[:, j, :],
                                             start=(c == 0), stop=(c == KC - 1))
                        c0 += gsz

                    # normalize: recip of denom row, broadcast via PE outer
                    # product (ones64 x recip), evict+scale on DVE
                    rc = small.tile([1, QB], F32R, tag="rc")
                    nc.vector.reciprocal(rc, outT[D:D + 1, :])
                    rb_ps = accps.tile([D, QB], F32, tag="acc")
                    nc.tensor.matmul(rb_ps, ones_sb[0:1, 0:D], rc[0:1, :],
                                     start=True, stop=True)
                    rb_sb = small.tile([D, QB], F32, tag="rb")
                    nc.vector.tensor_copy(rb_sb, rb_ps)
                    nc.vector.tensor_mul(
                        attnT[hh * D:hh * D + D, hp, :],
                        outT[0:D, :], rb_sb)

            # ---- output linear ----
            for qc in range(4):
                y_sb = ypool.tile([128, EMBED], F32)
                for n in range(2):
                    ps = accps.tile([128, 512], F32, tag="acc")
                    for kc in range(8):
                        nc.tensor.matmul(ps,
                                         attnT[:, kc, qc * 128:(qc + 1) * 128],
                                         WT_sb[:, kc, n * 512:(n + 1) * 512],
                                         start=(kc == 0), stop=(kc == 7))
                    nc.vector.tensor_add(y_sb[:, n * 512:(n + 1) * 512], ps,
                                         bias_bc[:, n * 512:(n + 1) * 512])
                nc.sync.dma_start(out=y[qc * 128:(qc + 1) * 128, :], in_=y_sb)

    nc.compile()
    return nc


def _prep_in_maps(q, k, v, padding_mask, W, b):
    q = np.asarray(q, dtype=np.float32)
    k = np.asarray(k, dtype=np.float32)
    v = np.asarray(v, dtype=np.float32)
    m = np.asarray(padding_mask)
    W = np.asarray(W, dtype=np.float32)
    b = np.asarray(b, dtype=np.float32)

    # [bs, seq, embed] -> [bs, heads, d, seq]
    qT = np.ascontiguousarray(q.reshape(BS, SEQ, HEADS, D).transpose(0, 2, 3, 1))
    kT = np.ascontiguousarray(k.reshape(BS, SEQ, HEADS, D).transpose(0, 2, 3, 1))
    # mask [bs, 1, q, k] -> float [bs, k, q]
    import ml_dtypes
    mT = np.ascontiguousarray(m[:, 0].transpose(0, 2, 1).astype(ml_dtypes.bfloat16))
    WTc = np.ascontiguousarray(W.T)

    in_maps = []
    for c in range(NC_COUNT):
        bi, qb = c // 4, c % 4
        in_maps.append({
            "qT": np.ascontiguousarray(qT[bi, :, :, qb * QB:(qb + 1) * QB]),
            "kT": kT[bi],
            "vb": v[bi],
            "mT": np.ascontiguousarray(mT[bi, :, qb * QB:(qb + 1) * QB]),
            "WT": WTc,
            "bias": b,
            "ones": np.ones((128, D), dtype=np.float32),
        })
    return in_maps


def _run(in_maps, **kw):
    if "nc" not in _CACHE:
        _CACHE["nc"] = _build_nc()
    return run_bass_kernel_spmd(_CACHE["nc"], in_maps, list(range(NC_COUNT)), **kw)


def kernel(q, k, v, padding_mask, W, b):
    in_maps = _prep_in_maps(q, k, v, padding_mask, W, b)
    res = _run(in_maps)
    out = np.empty((BS, SEQ, EMBED), dtype=np.float32)
    for c in range(NC_COUNT):
        bi, qb = c // 4, c % 4
        out[bi, qb * QB:(qb + 1) * QB] = res.results[c]["y"]
    return out



# revision 9
# speedup vs baseline: 1.0345x; 1.0345x over previous
"""Multi-head attention + output Linear on 8 Trainium2 NeuronCores.

Problem: bs=2, seq=2048, embed=1024, heads=16, head_dim=64.
  out = Linear(softmax(mask(Q K^T / 8)) V)        (eval-mode dropout)

Sharding: core c in 0..7 handles batch b = c//4 and query block qb = c%4
(512 query rows), computing its exact [512, 1024] output slice - heads stay
together per core so the output Linear needs no cross-core reduction.

Per-core kernel v2 (Tile framework), all matmul I/O in bf16 (fp32 PSUM):
  scoresT[k, q] = K_h^T Q_h            (PE, d=64 contraction)
  probsT = exp(scoresT / 8)            (ACT, PSUM->SBUF bf16; the bottleneck)
  probsT *= maskT                      (DVE bf16, 2x mode)
  pv[q, 0:64] = probsT_chunk^T V_chunk (PE flipped: probs chunk stationary,
                                        V chunk moving: 64+1 rows per chunk
                                        instead of 512 -> PV cost / 8)
  pv[q, 64]  = probsT_chunk^T ones     (softmax denominator)
  attn = pv[:, 0:64] * recip(den)      (DVE recip + Pool per-partition scale)
  attnT chunks via PE transpose        (identity matmul, 128 rows/block)
  y += attnT^T W^T per head-pair       (PE, accumulated into SBUF by Pool adds
                                        so there is no serial tail after the
                                        exp stream ends)

Issue order software-pipelines PV/transpose/linear of head h-1 into the
ACT-bound exp stream of head h.
"""

import sys
import numpy as np

sys.path.insert(0, "/opt/trn_rl_repo")

import concourse.bass as bass
import concourse.tile as tile
from concourse import bacc, mybir
from concourse.bass_utils import run_bass_kernel_spmd

BS, SEQ, EMBED, HEADS = 2, 2048, 1024, 16
D = EMBED // HEADS            # 64
QB = SEQ // 4                 # 512 query rows per core
NC_COUNT = 8
KC = SEQ // 128               # 16 k chunks
F32 = mybir.dt.float32
BF16 = mybir.dt.bfloat16

_CACHE = {}

# exp-group sizes per head (chunks per ACT call); sum must be KC
GROUPS = [(c0, 2) for c0 in range(0, KC, 2)]


def _build_nc():
    nc = bacc.Bacc("TRN2", target_bir_lowering=False, debug=False)

    qT = nc.dram_tensor("qT", [HEADS, D, QB], BF16, kind="ExternalInput")
    kT = nc.dram_tensor("kT", [HEADS, D, SEQ], BF16, kind="ExternalInput")
    v = nc.dram_tensor("v", [SEQ, EMBED], BF16, kind="ExternalInput")
    m = nc.dram_tensor("m", [SEQ, QB], BF16, kind="ExternalInput")
    WT = nc.dram_tensor("WT", [EMBED, EMBED], BF16, kind="ExternalInput")
    bias = nc.dram_tensor("bias", [EMBED], F32, kind="ExternalInput")
    ident = nc.dram_tensor("ident", [128, 128], BF16, kind="ExternalInput")
    y = nc.dram_tensor("y", [QB, EMBED], F32, kind="ExternalOutput")

    m_re = m.rearrange("(c p) q -> p c q", p=128)
    v_re = v.rearrange("(c p) e -> p c e", p=128)

    with tile.TileContext(nc) as tc, \
         nc.allow_low_precision(reason="bf16 matmul inputs; fp32 accumulate in PSUM"):
        with tc.tile_pool(name="const", bufs=1) as const, \
             tc.tile_pool(name="kpool", bufs=2) as kpool, \
             tc.tile_pool(name="probs", bufs=2) as probs, \
             tc.tile_pool(name="asb", bufs=2) as asbp, \
             tc.tile_pool(name="rcp", bufs=4) as rcp, \
             tc.tile_pool(name="scps", bufs=2, space="PSUM") as scps, \
             tc.tile_pool(name="auxps", bufs=2, space="PSUM") as auxps, \
             tc.tile_pool(name="linps", bufs=2, space="PSUM") as linps:

            # ---- constants / big inputs (DMA order = need order) ----
            kTp = [None] * 8
            kTp[0] = kpool.tile([128, SEQ], BF16, tag="kT", name="kTp0")
            nc.sync.dma_start(out=kTp[0],
                              in_=kT[0:2].rearrange("h d s -> (h d) s"))
            qT_sb = const.tile([128, 8, QB], BF16)
            nc.sync.dma_start(
                out=qT_sb,
                in_=qT.rearrange("(hp two) d q -> (two d) hp q", two=2))
            mT_sb = const.tile([128, KC, QB], BF16)
            nc.sync.dma_start(out=mT_sb[:, 0:4], in_=m_re[:, 0:4])
            vfull = const.tile([128, KC, EMBED], BF16)
            nc.sync.dma_start(out=vfull[:, 0:4], in_=v_re[:, 0:4])
            ident_sb = const.tile([128, 128], BF16)
            nc.sync.dma_start(out=ident_sb, in_=ident[:, :])
            nc.sync.dma_start(out=vfull[:, 4:8], in_=v_re[:, 4:8])
            nc.sync.dma_start(out=mT_sb[:, 4:8], in_=m_re[:, 4:8])
            nc.sync.dma_start(out=vfull[:, 8:KC], in_=v_re[:, 8:KC])
            nc.sync.dma_start(out=mT_sb[:, 8:KC], in_=m_re[:, 8:KC])
            WT_sb = const.tile([128, 8, EMBED], BF16)
            nc.sync.dma_start(out=WT_sb,
                              in_=WT.rearrange("(c p) e -> p c e", p=128))
            # y accumulator, initialized with broadcast bias
            y_acc = const.tile([128, 4, EMBED], F32)
            bias_ap = bias[:]
            nc.sync.dma_start(
                out=y_acc,
                in_=bass.AP(tensor=bias_ap.tensor, offset=bias_ap.offset,
                            ap=[[0, 128], [0, 4]] + list(bias_ap.ap)))
            ones_sb = const.tile([128, 1], BF16)
            nc.vector.memset(ones_sb, 1.0)
            attnT = const.tile([128, 8, QB], BF16)

            probs_t = {}
            pv_t = {}
            asb_t = {}

            def issue_pv(k, part):
                pk = probs_t[k]
                if part == 0:
                    pv_t[k] = auxps.tile([128, 4, D + 1], F32, tag="pv",
                                         name=f"pv{k}")
                    # 8 accumulation groups share this bank: a start=True
                    # would zero the whole bank, so init once and accumulate
                    nc.vector.memset(pv_t[k], 0.0)
                c_range = range(0, 8) if part == 0 else range(8, KC)
                for c in c_range:
                    for qb in range(4):
                        lhsT = pk[:, c, qb * 128:(qb + 1) * 128]
                        nc.tensor.matmul(
                            pv_t[k][:, qb, 0:D], lhsT,
                            vfull[:, c, k * D:(k + 1) * D],
                            start=False, stop=(c == KC - 1),
                            skip_group_check=True)
                        nc.tensor.matmul(
                            pv_t[k][:, qb, D:D + 1], lhsT,
                            ones_sb[:, 0:1],
                            start=False, stop=(c == KC - 1),
                            skip_group_check=True)

            def issue_norm(k):
                # reciprocal of denom col; rescale into transpose staging
                hp_k, hh_k = k // 2, k % 2
                if hh_k == 0:
                    asb_t[hp_k] = asbp.tile([128, 4, 2, D], BF16, tag="asb", name=f"asb{hp_k}")
                rc = rcp.tile([128, 4], F32, tag="rc", name=f"rc{k}")
                nc.vector.reciprocal(rc, pv_t[k][:, :, D])
                for qb in range(4):
                    nc.vector.tensor_scalar_mul(
                        asb_t[hp_k][:, qb, hh_k, :],
                        pv_t[k][:, qb, 0:D],
                        rc[:, qb:qb + 1])
                del pv_t[k]
                del probs_t[k]

            def issue_tr(hp_k):
                for qb in range(4):
                    trp = linps.tile([128, 128], BF16, tag="lin",
                                     name=f"tr{hp_k}_{qb}")
                    nc.tensor.transpose(trp, asb_t[hp_k][:, qb, :, :], ident_sb)
                    nc.vector.tensor_copy(
                        attnT[:, hp_k, qb * 128:(qb + 1) * 128], trp)
                del asb_t[hp_k]

            def issue_lin(qc, n, phase):
                lp = linps.tile([128, 512], F32, tag="lin",
                                name=f"lin{phase}_{qc}_{n}")
                for i, hpi in enumerate(range(4 * phase, 4 * phase + 4)):
                    nc.tensor.matmul(
                        lp,
                        attnT[:, hpi, qc * 128:(qc + 1) * 128],
                        WT_sb[:, hpi, n * 512:(n + 1) * 512],
                        start=(i == 0), stop=(i == 3))
                nc.vector.tensor_add(
                    y_acc[:, qc, n * 512:(n + 1) * 512], lp,
                    y_acc[:, qc, n * 512:(n + 1) * 512])

            pending_pv = None
            pending_tr = None

            for h in range(HEADS):
                hp, hh = h // 2, h % 2
                if hh == 0 and hp + 1 < 8:
                    kTp[hp + 1] = kpool.tile([128, SEQ], BF16, tag="kT",
                                              name=f"kTp{hp + 1}")
                    nc.sync.dma_start(
                        out=kTp[hp + 1],
                        in_=kT[2 * hp + 2:2 * hp + 4].rearrange(
                            "h d s -> (h d) s"))
                probs_t[h] = probs.tile([128, KC, QB], BF16, tag="probs", name=f"probs{h}")
                for gi, (c0, gsz) in enumerate(GROUPS):
                    sc = scps.tile([128, gsz, QB], F32, tag="sc")
                    for j in range(gsz):
                        c = c0 + j
                        nc.tensor.matmul(
                            sc[:, j, :],
                            kTp[hp][hh * D:(hh + 1) * D, c * 128:(c + 1) * 128],
                            qT_sb[hh * D:(hh + 1) * D, hp, :],
                            start=True, stop=True)
                    nc.scalar.activation(
                        out=probs_t[h][:, c0:c0 + gsz, :], in_=sc,
                        func=mybir.ActivationFunctionType.Exp,
                        scale=float(1.0 / np.sqrt(D)))
                    meng = nc.gpsimd if (h * len(GROUPS) + gi) % 6 == 5 \
                        else nc.vector
                    meng.tensor_mul(
                        probs_t[h][:, c0:c0 + gsz, :],
                        probs_t[h][:, c0:c0 + gsz, :],
                        mT_sb[:, c0:c0 + gsz, :])
                    if gi == 1 and pending_pv is not None:
                        issue_pv(pending_pv, 0)
                    elif gi == 3 and pending_pv is not None:
                        issue_pv(pending_pv, 1)
                    elif gi == 4 and pending_pv is not None:
                        issue_norm(pending_pv)
                    elif gi == 5 and pending_tr is not None:
                        issue_tr(pending_tr)
                        pending_tr = None
                    elif gi == 6 and h in (9, 10, 11, 12):
                        for n in range(2):
                            issue_lin(h - 9, n, 0)
                if pending_pv is not None and pending_pv % 2 == 1:
                    pending_tr = pending_pv // 2
                pending_pv = h

            # flush: PV + norm for head 15, transpose + linear phase B
            issue_pv(15, 0)
            issue_pv(15, 1)
            issue_norm(15)
            issue_tr(7)
            for qc in range(4):
                for n in range(2):
                    issue_lin(qc, n, 1)
            for qc in range(4):
                nc.sync.dma_start(out=y[qc * 128:(qc + 1) * 128, :],
                                  in_=y_acc[:, qc, :])

    nc.compile()
    return nc


def _prep_in_maps(q, k, v, padding_mask, W, b):
    import ml_dtypes
    bf = ml_dtypes.bfloat16
    q = np.asarray(q, dtype=np.float32)
    k = np.asarray(k, dtype=np.float32)
    v = np.asarray(v, dtype=np.float32)
    m = np.asarray(padding_mask)
    W = np.asarray(W, dtype=np.float32)
    b = np.asarray(b, dtype=np.float32)

    # [bs, seq, embed] -> [bs, heads, d, seq]
    qT = np.ascontiguousarray(
        q.reshape(BS, SEQ, HEADS, D).transpose(0, 2, 3, 1).astype(bf))
    kT = np.ascontiguousarray(
        k.reshape(BS, SEQ, HEADS, D).transpose(0, 2, 3, 1).astype(bf))
    vb = np.ascontiguousarray(v.astype(bf))
    # mask [bs, 1, q, k] -> [bs, k, q] in bf16 (0/1)
    mT = np.ascontiguousarray(m[:, 0].transpose(0, 2, 1).astype(bf))
    WTc = np.ascontiguousarray(W.T.astype(bf))
    ident = np.eye(128, dtype=bf)

    in_maps = []
    for c in range(NC_COUNT):
        bi, qb = c // 4, c % 4
        in_maps.append({
            "qT": np.ascontiguousarray(qT[bi, :, :, qb * QB:(qb + 1) * QB]),
            "kT": kT[bi],
            "v": vb[bi],
            "m": np.ascontiguousarray(mT[bi, :, qb * QB:(qb + 1) * QB]),
            "WT": WTc,
            "bias": b,
            "ident": ident,
        })
    return in_maps


def _run(in_maps, **kw):
    if "nc" not in _CACHE:
        _CACHE["nc"] = _build_nc()
    return run_bass_kernel_spmd(_CACHE["nc"], in_maps, list(range(NC_COUNT)), **kw)


def kernel(q, k, v, padding_mask, W, b):
    in_maps = _prep_in_maps(q, k, v, padding_mask, W, b)
    res = _run(in_maps)
    out = np.empty((BS, SEQ, EMBED), dtype=np.float32)
    for c in range(NC_COUNT):
        bi, qb = c // 4, c % 4
        out[bi, qb * QB:(qb + 1) * QB] = res.results[c]["y"]
    return out
